# revision 1
# baseline (speedup 1.0000x reference)
"""Bass/Trainium2 kernel for nn_BasicBlock_73933567033945 (CDConv / gnn_message_passing).

Strategy: graph is a fixed +-8 sequence window inside each of 4 chains of
L=2048 nodes (verified against the src/dst inputs at runtime). Shard the
8192 nodes across 8 NeuronCores (1024 nodes each, half a chain) with an
8-node halo; all gathers become partition shifts materialized by PE
shift-matmuls, the per-edge kernel is a PE matmul against a block-diagonal
weight, the kern (x) h bilinear runs as 17 broadcast multiplies on DVE, and
the (offset, channel) contraction runs on the PE via PSUM-accumulated
transposes followed by Wk-chunk matmuls. Pure data parallel: no collectives.
"""
import numpy as np

B, L, C = 4, 2048, 128
N = B * L
W = 32
KC = 24
SEQ_L = 11
R = 12.0
WIN = 8
NEG_IN = 0.1
NEG_K = 0.2
NCORES = 8
NPC = N // NCORES          # 1024 nodes per core
TS = 112                   # output nodes per tile
NT = 10                    # tiles per core (9*112 + 16)
HR = 9 * TS + 128          # 1136 halo rows per core
K17 = 2 * WIN + 1          # 17 window offsets
S_HALF = SEQ_L // 2

_PROG = {}


def _sidx(k):
    return int(np.clip(k - WIN, -S_HALF, S_HALF)) + S_HALF


def _build_program():
    import concourse.tile as tile
    from concourse import mybir, bacc
    from concourse.bass_utils import run_bass_kernel_spmd  # noqa: F401 (import check)
    from contextlib import ExitStack

    f32 = mybir.dt.float32
    AF = mybir.ActivationFunctionType
    OP = mybir.AluOpType
    AX = mybir.AxisListType

    nc = bacc.Bacc("TRN2", target_bir_lowering=False, debug=False)

    def din(name, shape):
        return nc.dram_tensor(name, shape, f32, kind="ExternalInput").ap()

    x_slot = din("x_slot", [128, NT * C])
    xc_slot = din("xc_slot", [128, NT * C])
    po_slot = din("po_slot", [128, NT * 12])
    w_in = din("w_in", [C, W])
    ws_a = din("ws_a", [128, K17 * KC])
    ws_b = din("ws_b", [8, K17 * KC])
    wk_p = din("wk_p", [128, 6 * W])
    w_out = din("w_out", [W, C])
    ident = din("ident", [128, 128])
    shifts = din("shifts", [128, K17 * TS])
    w5r = din("w5r", [128, 3 * KC])
    b5r = din("b5r", [128, KC])
    maskd = din("maskd", [128, NT * K17])
    nclmp = din("nclmp", [128, NT])
    alph1 = din("alph1", [128, 1])
    alph2 = din("alph2", [128, 1])
    y = nc.dram_tensor("y", [NPC, C], f32, kind="ExternalOutput").ap()

    with tile.TileContext(nc) as tc, ExitStack() as ctx:
        pers = ctx.enter_context(tc.tile_pool(name="pers", bufs=1))

        def load(ap_in, shape, tag):
            t = pers.tile(shape, f32, tag=tag)
            nc.sync.dma_start(t[:], ap_in)
            return t

        x_all = load(x_slot, [128, NT * C], "x_all")
        xc_all = load(xc_slot, [128, NT * C], "xc_all")
        # phys: per slot j, 44 cols [h(32) | pos(3) | ori(9)]
        phys = pers.tile([128, NT * 44], f32, tag="phys")
        nc.sync.dma_start(
            phys[:].rearrange("p (j c) -> p j c", c=44)[:, :, 32:44],
            po_slot.rearrange("p (j c) -> p j c", c=12))
        w_in_sb = load(w_in, [C, W], "w_in")
        ws_a_sb = load(ws_a, [128, K17 * KC], "ws_a")
        ws_b_sb = load(ws_b, [8, K17 * KC], "ws_b")
        wk_sb = load(wk_p, [128, 6 * W], "wk")
        w_out_sb = load(w_out, [W, C], "w_out")
        id_sb = load(ident, [128, 128], "ident")
        sh_sb = load(shifts, [128, K17 * TS], "shifts")
        w5r_sb = load(w5r, [128, 3 * KC], "w5r")
        b5r_sb = load(b5r, [128, KC], "b5r")
        mask_sb = load(maskd, [128, NT * K17], "mask")
        ncl_sb = load(nclmp, [128, NT], "nclmp")
        a1_sb = load(alph1, [128, 1], "a1")
        a2_sb = load(alph2, [128, 1], "a2")
        bf16 = mybir.dt.bfloat16
        idb_sb = pers.tile([128, 128], bf16, tag="identb")
        nc.vector.tensor_copy(idb_sb[:], id_sb[:])


        # ---------------- Phase A: h = lrelu(lrelu(x) @ W_in) per slot -----
        with tc.tile_pool(name="pA", bufs=2) as pA, \
             tc.tile_pool(name="pAp", bufs=2, space="PSUM") as pAp:
            for j in range(NT):
                xl = pA.tile([128, C], f32, tag="xl")
                nc.scalar.activation(xl[:], x_all[:, j * C:(j + 1) * C],
                                     AF.Prelu, bias=0.0, scale=1.0,
                                     alpha=a1_sb[:, 0:1])
                xT_p = pAp.tile([128, 128], f32, tag="xT")
                nc.tensor.matmul(xT_p[:], xl[:], id_sb[:], is_transpose=True,
                                 start=True, stop=True)
                xT = pA.tile([128, 128], f32, tag="xTsb")
                nc.scalar.copy(xT[:], xT_p[:])
                hp = pAp.tile([128, W], f32, tag="hp")
                nc.tensor.matmul(hp[:], xT[:], w_in_sb[:], start=True, stop=True)
                nc.scalar.activation(phys[:, 44 * j:44 * j + W], hp[:],
                                     AF.Prelu, bias=0.0, scale=1.0,
                                     alpha=a1_sb[:, 0:1])

        # ---------------- Phase B: per output tile ------------------------
        wrk = ctx.enter_context(tc.tile_pool(name="wrk", bufs=2))
        tpool = ctx.enter_context(tc.tile_pool(name="tmp", bufs=4))
        ps = ctx.enter_context(tc.tile_pool(name="ps", bufs=1, space="PSUM"))
        ps2 = ctx.enter_context(tc.tile_pool(name="ps2", bufs=2, space="PSUM"))

        P = TS  # 112 active partitions
        for t in range(NT):
            # ---- neighborhood materialization via shift matmuls ----------
            # nb layout: k-block (44 cols = h|pos|ori) at col(k); k<=10 in
            # bank0 (44k), k>=11 in bank1 (512+44(k-11)) so no matmul output
            # crosses a PSUM bank boundary.
            def ncol(k):
                return 44 * k if k <= 10 else 512 + 44 * (k - 11)
            nb_p = ps.tile([P, 820], f32, tag="nb")
            for k in range(K17):
                nc.tensor.matmul(nb_p[:, ncol(k):ncol(k) + 44],
                                 sh_sb[:, TS * k:TS * (k + 1)],
                                 phys[:, 44 * t:44 * t + 44],
                                 start=(k in (0, 11)), stop=(k in (10, 16)),
                                 skip_group_check=True)
            nb = wrk.tile([P, 820], f32, tag="nb_sb")
            nc.scalar.copy(nb[:, 0:776], nb_p[:, 0:776])
            pos_c = nb[:, ncol(8) + 32:ncol(8) + 35]       # k=8 center
            ori_c = nb[:, ncol(8) + 35:ncol(8) + 44]

            def kview(k0, kn, off, width):
                # [(k: step 44, kn), (c: step 1, width)] view at block k0+off
                return nb[:, ncol(k0) + off:ncol(k0) + off + 44 * kn] \
                    .rearrange("p (k c) -> p k c", c=44)[:, :, 0:width]

            # ---- geometry -> delta_aug [P, (k,8)] ------------------------
            da = wrk.tile([P, K17 * 8], f32, tag="da")
            dav = da[:].rearrange("p (k d) -> p k d", d=8)
            D = wrk.tile([P, K17 * 3], f32, tag="D")
            Dv = D[:].rearrange("p (k a) -> p k a", a=3)
            nc.vector.tensor_sub(
                Dv[:, 0:11, :], kview(0, 11, 32, 3),
                pos_c.unsqueeze(1).broadcast_to([P, 11, 3]))
            nc.vector.tensor_sub(
                Dv[:, 11:17, :], kview(11, 6, 32, 3),
                pos_c.unsqueeze(1).broadcast_to([P, 6, 3]))
            sq = wrk.tile([P, K17 * 3], f32, tag="sq")
            nc.vector.tensor_mul(sq[:], D[:], D[:])
            d2 = wrk.tile([P, K17], f32, tag="d2")
            nc.vector.tensor_reduce(d2[:], sq[:].rearrange("p (k a) -> p k a", a=3),
                                    axis=AX.X, op=OP.add)
            # dist/R into delta slot 6 ; raw dist for direction
            nc.scalar.activation(dav[:, :, 6], d2[:], AF.Sqrt, bias=0.0,
                                 scale=1.0 / (R * R))
            dist = wrk.tile([P, K17], f32, tag="dist")
            nc.scalar.activation(dist[:], d2[:], AF.Sqrt, bias=0.0, scale=1.0)
            rec = wrk.tile([P, K17], f32, tag="rec")
            nc.vector.tensor_scalar_add(dist[:], dist[:], 1e-9)
            nc.vector.reciprocal(rec[:], dist[:])
            dirn = wrk.tile([P, K17 * 3], f32, tag="dirn")
            dirnv = dirn[:].rearrange("p (k a) -> p k a", a=3)
            nc.vector.tensor_mul(dirnv, Dv,
                                 rec[:].unsqueeze(-1).broadcast_to([P, K17, 3]))
            # local_a = sum_b Ri[a,b] * dirn[b]
            lm = wrk.tile([P, K17 * 9], f32, tag="lm")
            lmv = lm[:].rearrange("p (k a b) -> p k a b", a=3, b=3)
            nc.vector.tensor_mul(
                lmv,
                ori_c.rearrange("p (a b) -> p a b", b=3).unsqueeze(1)
                     .broadcast_to([P, K17, 3, 3]),
                dirn[:].rearrange("p (k b) -> p k b", b=3).unsqueeze(2)
                       .broadcast_to([P, K17, 3, 3]))
            nc.vector.tensor_reduce(dav[:, :, 0:3], lmv, axis=AX.X, op=OP.add)
            # ofeat_a = sum_b Ri[a,b] * Rj[a,b]
            ofm = wrk.tile([P, K17 * 9], f32, tag="ofm")
            ofmv = ofm[:].rearrange("p (k a b) -> p k a b", a=3, b=3)
            nc.vector.tensor_mul(
                ofmv[:, 0:11],
                kview(0, 11, 35, 9).rearrange("p k (a b) -> p k a b", b=3),
                ori_c.rearrange("p (a b) -> p a b", b=3).unsqueeze(1)
                     .broadcast_to([P, 11, 3, 3]))
            nc.vector.tensor_mul(
                ofmv[:, 11:17],
                kview(11, 6, 35, 9).rearrange("p k (a b) -> p k a b", b=3),
                ori_c.rearrange("p (a b) -> p a b", b=3).unsqueeze(1)
                     .broadcast_to([P, 6, 3, 3]))
            nc.vector.tensor_reduce(dav[:, :, 3:6], ofmv, axis=AX.X, op=OP.add)
            nc.vector.memset(dav[:, :, 7], 1.0)
            # chain-boundary mask (zeroes whole delta rows incl. bias)
            nc.vector.tensor_mul(
                dav, dav,
                mask_sb[0:P, K17 * t:K17 * (t + 1)].unsqueeze(-1)
                      .broadcast_to([P, K17, 8]))

            # ---- kern = lrelu(delta_aug @ WS, 0.2) -----------------------
            dT_p = ps.tile([128, 224], f32, tag="dT")
            nc.tensor.matmul(dT_p[:, 0:P], da[:, 0:128], id_sb[0:P, 0:P],
                             is_transpose=True, start=True, stop=False,
                             skip_group_check=True)
            nc.tensor.matmul(dT_p[0:8, P:P + P], da[:, 128:136], id_sb[0:P, 0:P],
                             is_transpose=True, start=False, stop=True,
                             skip_group_check=True)
            dT = wrk.tile([128, 224], f32, tag="dT_sb")
            nc.scalar.copy(dT[:], dT_p[:])
            pre_p = ps.tile([P, K17 * KC], f32, tag="pre")
            nc.tensor.matmul(pre_p[:], dT[:, 0:P], ws_a_sb[:], start=True,
                             stop=False, skip_group_check=True)
            nc.tensor.matmul(pre_p[:], dT[0:8, P:P + P], ws_b_sb[:], start=False,
                             stop=True, skip_group_check=True)
            kern = wrk.tile([P, K17 * KC], f32, tag="kern")
            nc.scalar.activation(kern[:], pre_p[:], AF.Prelu, bias=0.0,
                                 scale=1.0, alpha=a2_sb[0:P, 0:1])

            # ---- self-edge compensation into kern k=8 block --------------
            rn = wrk.tile([P, 3], f32, tag="rn")
            nc.vector.tensor_reduce(
                rn[:], ofm[:, 72:81].rearrange("p (a b) -> p a b", b=3),
                axis=AX.X, op=OP.add)
            pself = wrk.tile([P, KC], f32, tag="pself")
            nc.vector.scalar_tensor_tensor(pself[:], w5r_sb[0:P, 0:KC],
                                           rn[:, 0:1], b5r_sb[0:P, :],
                                           OP.mult, OP.add)
            nc.vector.scalar_tensor_tensor(pself[:], w5r_sb[0:P, KC:2 * KC],
                                           rn[:, 1:2], pself[:], OP.mult, OP.add)
            nc.vector.scalar_tensor_tensor(pself[:], w5r_sb[0:P, 2 * KC:3 * KC],
                                           rn[:, 2:3], pself[:], OP.mult, OP.add)
            kself = wrk.tile([P, KC], f32, tag="kself")
            nc.vector.scalar_tensor_tensor(kself[:], pself[:], NEG_K, pself[:],
                                           OP.mult, OP.max)
            nc.vector.tensor_scalar_mul(kself[:], kself[:], ncl_sb[0:P, t:t + 1])
            nc.gpsimd.tensor_add(kern[:, 8 * KC:9 * KC],
                                 kern[:, 8 * KC:9 * KC], kself[:])

            # ---- bilinear: tmp_k = kern_k (x) h_shift_k; PE transp-accum -
            aggT_p = ps.tile([128, 768], f32, tag="aggT")

            def tmp_mult(k, tag, eng):
                tm = tpool.tile([P, KC * W], bf16, tag=tag)
                eng.tensor_mul(
                    tm[:].rearrange("p (c w) -> p c w", w=W),
                    nb[:, ncol(k):ncol(k) + W].unsqueeze(1)
                      .broadcast_to([P, KC, W]),
                    kern[:, KC * k:KC * (k + 1)].unsqueeze(-1)
                        .broadcast_to([P, KC, W]))
                return tm

            def tmp_transp(k, tm):
                for b in range(6):
                    nc.tensor.matmul(
                        aggT_p[:, 128 * b:128 * b + P],
                        tm[:, 128 * b:128 * (b + 1)], idb_sb[0:P, 0:P],
                        start=(k == 0 and b in (0, 4)),
                        stop=(k == 16 and b in (3, 5)),
                        skip_group_check=True)

            for k in range(K17):
                tmp_transp(k, tmp_mult(k, "tmp", nc.vector))
            aggT = wrk.tile([128, 768], f32, tag="aggT_sb")
            nc.scalar.copy(aggT[:], aggT_p[:])

            # ---- conv = lrelu(agg @ Wk, 0.1) ; out = conv @ W_out + x ----
            co_p = ps2.tile([P, 240], f32, tag="co")
            for b in range(6):
                nc.tensor.matmul(co_p[0:W, 0:P], wk_sb[:, W * b:W * (b + 1)],
                                 aggT[:, 128 * b:128 * b + P],
                                 start=(b == 0), stop=(b == 5),
                                 skip_group_check=True)
            convL = wrk.tile([W, P], f32, tag="convL")
            nc.scalar.activation(convL[:], co_p[0:W, 0:P], AF.Prelu, bias=0.0,
                                 scale=1.0, alpha=a1_sb[0:W, 0:1])
            # start=True: zeroes this bank on partitions 0..111 (convT results
            # already consumed by the Prelu above; zeroing is per-partition-range)
            nc.tensor.matmul(co_p[:, P:P + 128], convL[:], w_out_sb[:],
                             start=True, stop=True, skip_group_check=True)
            out_sb = wrk.tile([P, C], f32, tag="out_sb")
            nc.vector.tensor_add(out_sb[:], co_p[:, P:P + 128],
                                 xc_all[0:P, C * t:C * t + C])
            cnt = min(TS, NPC - TS * t)
            nc.sync.dma_start(y[TS * t:TS * t + cnt, :], out_sb[0:cnt, :])

    nc.compile()
    return nc


def _expected_src_dst():
    i = np.arange(N)
    offs = np.arange(-WIN, WIN + 1)
    j = i[:, None] + offs[None, :]
    valid = ((j // L) == (i[:, None] // L)) & (j >= 0) & (j < N)
    j = np.where(valid, j, i[:, None])
    dst = np.repeat(i, offs.size).astype(np.int32)
    src = j.reshape(-1).astype(np.int32)
    return src, dst


def _host_inputs(x, pos, ori, W_in, Ws0, bs0, Wk, W_out):
    xf = np.ascontiguousarray(x.reshape(N, C), np.float32)
    pos = np.asarray(pos, np.float32)
    ori = np.asarray(ori, np.float32)

    # shared weights / constants
    WS = np.zeros((136, K17 * KC), np.float32)
    for k in range(K17):
        s = _sidx(k)
        WS[8 * k:8 * k + 7, KC * k:KC * (k + 1)] = Ws0[s]
        WS[8 * k + 7, KC * k:KC * (k + 1)] = bs0[s]
    wk_p = np.zeros((128, 6 * W), np.float32)
    for b in range(6):
        wk_p[:, W * b:W * (b + 1)] = Wk[128 * b:128 * (b + 1), :]
    shifts = np.zeros((128, K17 * TS), np.float32)
    for k in range(K17):
        for p in range(TS):
            shifts[p + k, TS * k + p] = 1.0
    w5r = np.tile(Ws0[5][3:6].reshape(1, 3 * KC), (128, 1)).astype(np.float32)
    b5r = np.tile(bs0[5].reshape(1, KC), (128, 1)).astype(np.float32)
    common = dict(
        w_in=np.ascontiguousarray(W_in, np.float32),
        ws_a=np.ascontiguousarray(WS[0:128]),
        ws_b=np.ascontiguousarray(WS[128:136]),
        wk_p=wk_p,
        w_out=np.ascontiguousarray(W_out, np.float32),
        ident=np.eye(128, dtype=np.float32),
        shifts=shifts,
        w5r=w5r, b5r=b5r,
        alph1=np.full((128, 1), NEG_IN, np.float32),
        alph2=np.full((128, 1), NEG_K, np.float32),
    )

    in_maps = []
    for ci in range(NCORES):
        s0 = ci * NPC
        g = s0 - WIN + np.arange(HR)
        ok = (g >= 0) & (g < N)
        gi = np.clip(g, 0, N - 1)
        x_pad = np.where(ok[:, None], xf[gi], 0.0).astype(np.float32)
        p_pad = np.where(ok[:, None], pos[gi], 0.0).astype(np.float32)
        o_pad = np.where(ok[:, None], ori[gi], 0.0).astype(np.float32)

        jj, pp = np.meshgrid(np.arange(NT), np.arange(128), indexing="ij")
        rows = (TS * jj + pp)            # [NT, 128] all < HR
        x_slot = x_pad[rows].transpose(1, 0, 2).reshape(128, NT * C)
        po_pad = np.concatenate([p_pad, o_pad], axis=1)  # [HR, 12]
        po_slot = po_pad[rows].transpose(1, 0, 2).reshape(128, NT * 12)
        rc = WIN + TS * jj + pp
        okc = rc < HR
        xc_slot = np.where(okc[:, :, None], x_pad[np.minimum(rc, HR - 1)], 0.0)
        xc_slot = xc_slot.transpose(1, 0, 2).reshape(128, NT * C).astype(np.float32)

        mask = np.zeros((128, NT, K17), np.float32)
        ncl = np.zeros((128, NT), np.float32)
        for t in range(NT):
            for p in range(min(TS, NPC - TS * t) if TS * t < NPC else 0):
                n = s0 + TS * t + p
                off = n % L
                v = ((off + np.arange(-WIN, WIN + 1)) >= 0) & \
                    ((off + np.arange(-WIN, WIN + 1)) < L)
                mask[p, t, :] = v.astype(np.float32)
                ncl[p, t] = K17 - v.sum()
        in_maps.append(dict(
            x_slot=x_slot, xc_slot=xc_slot, po_slot=po_slot,
            maskd=mask.reshape(128, NT * K17), nclmp=ncl, **common))
    return in_maps


def kernel(x, pos, seq, ori, W_in, Ws0, bs0, Wk, W_out, src, dst):
    exp_src, exp_dst = _expected_src_dst()
    assert np.array_equal(np.asarray(src), exp_src), "unexpected src graph"
    assert np.array_equal(np.asarray(dst), exp_dst), "unexpected dst graph"

    from concourse.bass_utils import run_bass_kernel_spmd

    if "nc" not in _PROG:
        _PROG["nc"] = _build_program()
    nc = _PROG["nc"]

    in_maps = _host_inputs(np.asarray(x), np.asarray(pos), np.asarray(ori),
                           np.asarray(W_in), np.asarray(Ws0), np.asarray(bs0),
                           np.asarray(Wk), np.asarray(W_out))
    res = run_bass_kernel_spmd(nc, in_maps, list(range(NCORES)))
    out = np.concatenate([res.results[i]["y"] for i in range(NCORES)], axis=0)
    return out.reshape(B, L, C).astype(np.float32)



# revision 14
# speedup vs baseline: 1.9588x; 1.9588x over previous
"""Bass/Trainium2 kernel for nn_BasicBlock_73933567033945 (CDConv / gnn_message_passing).

v2 strategy (graph = fixed +-8 sequence window inside 4 chains, verified at
runtime): shard 8192 nodes across 8 cores (1024 each, half a chain), slot
layout of 128-row halo windows at stride 112.  All matmuls and DVE tensor ops
run in fp16 (fp32 PSUM accumulation); pos is slot-centered on host so fp16
holds precision.  The 17 window shifts are materialized once per core by 17
wide shift-matmuls over all 10 slots (h|pos|ori, 440 cols each).  The
per-edge kernel MLP output is written pair-duplicated (kern2) so the
bilinear kern (x) h product runs in the DVE 2x perf mode.  The (offset,
channel) contraction runs on the PE via PSUM-accumulated transposes followed
by Wk-chunk matmuls, all fp16.  Pure data parallel: no collectives.
"""
import numpy as np

B, L, C = 4, 2048, 128
N = B * L
W = 32
KC = 24
SEQ_L = 11
R = 12.0
WIN = 8
NEG_IN = 0.1
NEG_K = 0.2
NCORES = 8
NPC = N // NCORES          # 1024 nodes per core
TS = 112                   # output nodes per tile
NT = 10                    # tiles per core (9*112 + 16)
HR = 9 * TS + 128          # 1136 halo rows per core
K17 = 2 * WIN + 1          # 17 window offsets
S_HALF = SEQ_L // 2
PH = 44                    # phys cols per slot: h(32) | pos(3) | ori(9)
NBW = NT * PH              # 440: NB cols per k

_PROG = {}


def _sidx(k):
    return int(np.clip(k - WIN, -S_HALF, S_HALF)) + S_HALF


def _build_program():
    import concourse.tile as tile
    from concourse import mybir, bacc
    from concourse.bass_utils import run_bass_kernel_spmd  # noqa: F401 (import check)
    from contextlib import ExitStack

    f32 = mybir.dt.float32
    f16 = mybir.dt.float16
    AF = mybir.ActivationFunctionType
    OP = mybir.AluOpType
    AX = mybir.AxisListType

    nc = bacc.Bacc("TRN2", target_bir_lowering=False, debug=False)

    def din(name, shape, dt=f16):
        return nc.dram_tensor(name, shape, dt, kind="ExternalInput").ap()

    xT_slot = din("xT_slot", [128, NT * 128], f32)   # x transposed per slot
    xc_slot = din("xc_slot", [128, NT * C], f32)     # identity (center rows)
    pos_slot = din("pos_slot", [128, NT * 3])        # centered fp16 pos
    ori_slot = din("ori_slot", [128, NT * 9])
    w_in = din("w_in", [C, W])
    ws2a = din("ws2a", [128, 2 * K17 * KC])
    ws2b = din("ws2b", [8, 2 * K17 * KC])
    wk_p = din("wk_p", [128, 6 * W])
    w_out = din("w_out", [W, C])
    ident = din("ident", [128, 128])
    shifts = din("shifts", [128, K17 * TS])
    maskd = din("maskd", [128, NT * K17])
    kself2 = din("kself2", [128, NT * 2 * KC])
    y = nc.dram_tensor("y", [NPC, C], f32, kind="ExternalOutput").ap()

    P = TS  # 112 active partitions

    with tile.TileContext(nc) as tc, ExitStack() as ctx:
        pers = ctx.enter_context(tc.tile_pool(name="pers", bufs=1))

        def load(ap_in, shape, tag, dt=f16):
            t = pers.tile(shape, dt, tag=tag)
            nc.sync.dma_start(t[:], ap_in)
            return t

        xT_all = load(xT_slot, [128, NT * 128], "xT_all", f32)
        xc_all = load(xc_slot, [128, NT * C], "xc_all", f32)
        w_in_sb = load(w_in, [C, W], "w_in")
        ws2a_sb = load(ws2a, [128, 2 * K17 * KC], "ws2a")
        ws2b_sb = load(ws2b, [8, 2 * K17 * KC], "ws2b")
        wk_sb = load(wk_p, [128, 6 * W], "wk")
        w_out_sb = load(w_out, [W, C], "w_out")
        id_sb = load(ident, [128, 128], "ident")
        sh_sb = load(shifts, [128, K17 * TS], "shifts")
        mask_sb = load(maskd, [128, NT * K17], "mask")
        ks2_sb = load(kself2, [128, NT * 2 * KC], "kself2")

        # dist = sqrt(d2 + eps): eps = 1e-4 keeps rec = 1/dist <= 100 (fp16
        # safe; self-edges have D = 0 so local = 0 regardless) while real
        # edge distances (>= ~0.5) are perturbed by < 1e-3 relative.
        eps_sb = pers.tile([128, 1], f32, tag="eps")
        nc.vector.memset(eps_sb[:], 1e-4)

        # phys: per slot j, 44 cols [h(32) | pos(3) | ori(9)], all fp16
        phys = pers.tile([128, NBW], f16, tag="phys")
        nc.sync.dma_start(
            phys[:].rearrange("p (j c) -> p j c", c=PH)[:, :, 32:35],
            pos_slot.rearrange("p (j c) -> p j c", c=3))
        nc.sync.dma_start(
            phys[:].rearrange("p (j c) -> p j c", c=PH)[:, :, 35:44],
            ori_slot.rearrange("p (j c) -> p j c", c=9))

        # ---------------- Phase A: h = lrelu(lrelu(x) @ W_in) per slot -----
        with tc.tile_pool(name="pA", bufs=2) as pA, \
             tc.tile_pool(name="pAp", bufs=2, space="PSUM") as pAp:
            for j in range(NT):
                xlT = pA.tile([128, 128], f16, tag="xlT")
                nc.scalar.activation(xlT[:], xT_all[:, 128 * j:128 * (j + 1)],
                                     AF.Prelu, bias=0.0, scale=1.0, alpha=NEG_IN)
                hp = pAp.tile([128, W], f32, tag="hp")
                nc.tensor.matmul(hp[:], xlT[:], w_in_sb[:], start=True, stop=True)
                nc.scalar.activation(phys[:, PH * j:PH * j + W], hp[:],
                                     AF.Prelu, bias=0.0, scale=1.0, alpha=NEG_IN)

        # ---------------- Phase NB: 17 shift matmuls over all slots --------
        NB = pers.tile([P, K17 * NBW], f16, tag="NB")
        with tc.tile_pool(name="pNB", bufs=2, space="PSUM") as pNB:
            for k in range(K17):
                nb_p = pNB.tile([P, NBW], f32, tag="nb_p")
                nc.tensor.matmul(nb_p[:], sh_sb[:, TS * k:TS * (k + 1)],
                                 phys[:], start=True, stop=True)
                nc.scalar.copy(NB[:, NBW * k:NBW * (k + 1)], nb_p[:])

        def nbv(k, t, off, width):
            return NB[:, NBW * k + PH * t + off:NBW * k + PH * t + off + width]

        # ---------------- Phase B: per output tile ------------------------
        wrk = ctx.enter_context(tc.tile_pool(name="wrk", bufs=2))
        tpool = ctx.enter_context(tc.tile_pool(name="tmp", bufs=4))
        psA = ctx.enter_context(tc.tile_pool(name="psA", bufs=1, space="PSUM"))
        psD = ctx.enter_context(tc.tile_pool(name="psD", bufs=2, space="PSUM"))
        psP = ctx.enter_context(tc.tile_pool(name="psP", bufs=1, space="PSUM"))
        psC = ctx.enter_context(tc.tile_pool(name="psC", bufs=2, space="PSUM"))

        for t in range(NT):
            # k-strided views into NB for slot t
            def kview(off, width):
                # [P, K17, width] with k stride NBW
                v = NB[:].rearrange("p (k j) -> p k j", j=NBW)
                return v[:, :, PH * t + off:PH * t + off + width]

            pos_c = nbv(8, t, 32, 3)        # [P, 3] center pos
            ori_c = nbv(8, t, 35, 9)        # [P, 9] center frame

            # ---- geometry -> dav [P, (k,8)] fp16 -------------------------
            D = wrk.tile([P, K17 * 3], f16, tag="D")
            Dv = D[:].rearrange("p (k a) -> p k a", a=3)
            nc.vector.tensor_sub(Dv, kview(32, 3),
                                 pos_c.unsqueeze(1).broadcast_to([P, K17, 3]))
            sq = wrk.tile([P, K17 * 3], f16, tag="sq")
            nc.vector.tensor_mul(sq[:], D[:], D[:])
            d2 = wrk.tile([P, K17], f32, tag="d2")
            nc.vector.tensor_reduce(d2[:], sq[:].rearrange("p (k a) -> p k a", a=3),
                                    axis=AX.X, op=OP.add)
            dav = wrk.tile([P, K17 * 8], f16, tag="dav")
            davv = dav[:].rearrange("p (k d) -> p k d", d=8)
            # dist/R into delta slot 6 (sqrt(d2)/R)
            nc.scalar.activation(davv[:, :, 6], d2[:], AF.Sqrt, bias=0.0,
                                 scale=1.0 / (R * R))
            dist = wrk.tile([P, K17], f32, tag="dist")
            nc.scalar.activation(dist[:], d2[:], AF.Sqrt, bias=eps_sb[0:P, 0:1],
                                 scale=1.0)
            rec = wrk.tile([P, K17], f16, tag="rec")
            with nc.allow_low_precision(reason="fp16 direction scale is ok"):
                nc.vector.reciprocal(rec[:], dist[:])
            # local_a = (sum_b Ri[a,b] * D[k,b]) * rec[k]
            lm = wrk.tile([P, K17 * 9], f16, tag="lm")
            lmv = lm[:].rearrange("p (k a b) -> p k a b", a=3, b=3)
            nc.vector.tensor_mul(
                lmv,
                ori_c.rearrange("p (a b) -> p a b", b=3).unsqueeze(1)
                     .broadcast_to([P, K17, 3, 3]),
                D[:].rearrange("p (k b) -> p k b", b=3).unsqueeze(2)
                    .broadcast_to([P, K17, 3, 3]))
            locr = wrk.tile([P, K17 * 3], f16, tag="locr")
            with nc.allow_low_precision(reason="3-term sums, fp16 ok"):
                nc.vector.tensor_reduce(
                    locr[:].rearrange("p (k a) -> p k a", a=3), lmv,
                    axis=AX.X, op=OP.add)
            nc.vector.tensor_mul(
                davv[:, :, 0:3], locr[:].rearrange("p (k a) -> p k a", a=3),
                rec[:].unsqueeze(-1).broadcast_to([P, K17, 3]))
            # ofeat_a = sum_b Ri[a,b] * Rj[a,b]
            ofm = wrk.tile([P, K17 * 9], f16, tag="ofm")
            nc.vector.tensor_mul(
                ofm[:].rearrange("p (k e) -> p k e", e=9), kview(35, 9),
                ori_c.unsqueeze(1).broadcast_to([P, K17, 9]))
            with nc.allow_low_precision(reason="3-term sums, fp16 ok"):
                nc.vector.tensor_reduce(
                    davv[:, :, 3:6],
                    ofm[:].rearrange("p (k a b) -> p k a b", a=3, b=3),
                    axis=AX.X, op=OP.add)
            nc.vector.memset(davv[:, :, 7], 1.0)
            # chain-boundary mask (zeroes whole delta rows incl. bias)
            nc.vector.tensor_mul(
                davv, davv,
                mask_sb[0:P, K17 * t:K17 * (t + 1)].unsqueeze(-1)
                      .broadcast_to([P, K17, 8]))

            # ---- kern2 = lrelu(dav @ WS2, 0.2), pair-duplicated ----------
            dT_p = psD.tile([128, 224], f16, tag="dT")
            nc.tensor.matmul(dT_p[:, 0:P], dav[:, 0:128], id_sb[0:P, 0:P],
                             is_transpose=True, start=True, stop=False,
                             skip_group_check=True)
            nc.tensor.matmul(dT_p[0:8, P:P + P], dav[:, 128:136], id_sb[0:P, 0:P],
                             is_transpose=True, start=False, stop=True,
                             skip_group_check=True)
            dT = wrk.tile([128, 224], f16, tag="dT_sb")
            nc.scalar.copy(dT[:], dT_p[:])
            W2 = 2 * K17 * KC  # 816
            # psum banks are 512 f32 cols: put k-blocks 0..9 at 0:480 (bank 0)
            # and k-blocks 10..16 at 512:848 (bank 1) to avoid bank crossing.
            pre_p = psP.tile([P, 848], f32, tag="pre")
            nc.tensor.matmul(pre_p[:, 0:480], dT[:, 0:P], ws2a_sb[:, 0:480],
                             start=True, stop=False, skip_group_check=True)
            nc.tensor.matmul(pre_p[:, 512:848], dT[:, 0:P], ws2a_sb[:, 480:W2],
                             start=True, stop=False, skip_group_check=True)
            nc.tensor.matmul(pre_p[:, 0:480], dT[0:8, P:P + P], ws2b_sb[:, 0:480],
                             start=False, stop=True, skip_group_check=True)
            nc.tensor.matmul(pre_p[:, 512:848], dT[0:8, P:P + P], ws2b_sb[:, 480:W2],
                             start=False, stop=True, skip_group_check=True)
            kern2 = wrk.tile([P, W2], f16, tag="kern2")
            nc.scalar.activation(kern2[:, 0:480], pre_p[:, 0:480], AF.Prelu,
                                 bias=0.0, scale=1.0, alpha=NEG_K)
            nc.scalar.activation(kern2[:, 480:W2], pre_p[:, 512:848], AF.Prelu,
                                 bias=0.0, scale=1.0, alpha=NEG_K)
            # self-edge compensation (host-precomputed, pair-duplicated)
            K8 = 2 * KC * 8
            nc.vector.tensor_add(kern2[:, K8:K8 + 2 * KC],
                                 kern2[:, K8:K8 + 2 * KC],
                                 ks2_sb[0:P, 2 * KC * t:2 * KC * (t + 1)])

            # ---- bilinear + PE transpose-accumulate ----------------------
            aggT_p = psA.tile([128, 768], f32, tag="aggT")
            for k in range(K17):
                tm = tpool.tile([P, KC * W], f16, tag="tm")
                hv = nbv(k, t, 0, 32).rearrange("p (s two) -> p s two", two=2) \
                    .unsqueeze(1).broadcast_to([P, KC, 16, 2])
                kv = kern2[:, 2 * KC * k:2 * KC * (k + 1)] \
                    .rearrange("p (c two) -> p c two", two=2) \
                    .unsqueeze(2).broadcast_to([P, KC, 16, 2])
                nc.vector.tensor_tensor(
                    tm[:].rearrange("p (c s two) -> p c s two", two=2, s=16),
                    hv, kv, op=OP.mult)
                for b in range(6):
                    nc.tensor.matmul(
                        aggT_p[:, 128 * b:128 * b + P],
                        tm[:, 128 * b:128 * (b + 1)], id_sb[0:P, 0:P],
                        start=(k == 0 and b in (0, 4)),
                        stop=(k == 16 and b in (3, 5)),
                        skip_group_check=True)
            aggT = wrk.tile([128, 768], f16, tag="aggT_sb")
            nc.scalar.copy(aggT[:], aggT_p[:])

            # ---- conv = lrelu(agg @ Wk, 0.1) ; out = conv @ W_out + x ----
            co_p = psC.tile([P, 240], f32, tag="co")
            for b in range(6):
                nc.tensor.matmul(co_p[0:W, 0:P], wk_sb[:, W * b:W * (b + 1)],
                                 aggT[:, 128 * b:128 * b + P],
                                 start=(b == 0), stop=(b == 5),
                                 skip_group_check=True)
            convL = wrk.tile([W, P], f16, tag="convL")
            nc.scalar.activation(convL[:], co_p[0:W, 0:P], AF.Prelu, bias=0.0,
                                 scale=1.0, alpha=NEG_IN)
            nc.tensor.matmul(co_p[:, P:P + 128], convL[:], w_out_sb[:],
                             start=True, stop=True, skip_group_check=True)
            out_sb = wrk.tile([P, C], f32, tag="out_sb")
            nc.vector.tensor_add(out_sb[:], co_p[:, P:P + 128],
                                 xc_all[0:P, C * t:C * t + C])
            cnt = min(TS, NPC - TS * t)
            nc.sync.dma_start(y[TS * t:TS * t + cnt, :], out_sb[0:cnt, :])

    nc.compile()
    return nc


def _expected_src_dst():
    i = np.arange(N)
    offs = np.arange(-WIN, WIN + 1)
    j = i[:, None] + offs[None, :]
    valid = ((j // L) == (i[:, None] // L)) & (j >= 0) & (j < N)
    j = np.where(valid, j, i[:, None])
    dst = np.repeat(i, offs.size).astype(np.int32)
    src = j.reshape(-1).astype(np.int32)
    return src, dst


def _host_inputs(x, pos, ori, W_in, Ws0, bs0, Wk, W_out):
    xf = np.ascontiguousarray(x.reshape(N, C), np.float32)
    pos = np.asarray(pos, np.float32)
    ori = np.asarray(ori, np.float32)
    f16 = np.float16

    # shared weights / constants
    WS = np.zeros((136, K17 * KC), np.float32)
    for k in range(K17):
        s = _sidx(k)
        WS[8 * k:8 * k + 7, KC * k:KC * (k + 1)] = Ws0[s]
        WS[8 * k + 7, KC * k:KC * (k + 1)] = bs0[s]
    # pair-duplicate columns: WS2[:, 48k + 2c + j] = WS[:, 24k + c]
    WS2 = np.repeat(WS, 2, axis=1)
    wk_p = np.zeros((128, 6 * W), np.float32)
    for b in range(6):
        wk_p[:, W * b:W * (b + 1)] = Wk[128 * b:128 * (b + 1), :]
    shifts = np.zeros((128, K17 * TS), np.float32)
    for k in range(K17):
        for p in range(TS):
            shifts[p + k, TS * k + p] = 1.0
    common = dict(
        w_in=W_in.astype(f16),
        ws2a=WS2[0:128].astype(f16),
        ws2b=WS2[128:136].astype(f16),
        wk_p=wk_p.astype(f16),
        w_out=W_out.astype(f16),
        ident=np.eye(128, dtype=f16),
        shifts=shifts.astype(f16),
    )

    # self-edge compensation: kself[n] = lrelu(rn @ W5[3:6] + b5, 0.2) * ncl
    rn = (ori.reshape(N, 3, 3) ** 2).sum(axis=2)          # [N, 3]
    pself = rn @ np.asarray(Ws0[S_HALF][3:6], np.float32) \
        + np.asarray(bs0[S_HALF], np.float32)             # [N, KC]
    kself_full = np.where(pself >= 0, pself, NEG_K * pself)

    in_maps = []
    for ci in range(NCORES):
        s0 = ci * NPC
        g = s0 - WIN + np.arange(HR)
        ok = (g >= 0) & (g < N)
        gi = np.clip(g, 0, N - 1)
        x_pad = np.where(ok[:, None], xf[gi], 0.0).astype(np.float32)
        p_pad = np.where(ok[:, None], pos[gi], 0.0).astype(np.float32)
        o_pad = np.where(ok[:, None], ori[gi], 0.0).astype(np.float32)

        jj, pp = np.meshgrid(np.arange(NT), np.arange(128), indexing="ij")
        rows = (TS * jj + pp)            # [NT, 128] all < HR
        # xT_slot: [128(c), (t, p)] transposed slots
        x_sl = x_pad[rows]               # [NT, 128, C]
        xT_slot = np.ascontiguousarray(
            x_sl.transpose(2, 0, 1).reshape(C, NT * 128), np.float32)
        # pos: center per slot for fp16 precision
        p_sl = p_pad[rows]               # [NT, 128, 3]
        ctr = p_sl.mean(axis=1, keepdims=True)
        p_sl = (p_sl - ctr).astype(f16)
        pos_slot = np.ascontiguousarray(
            p_sl.transpose(1, 0, 2).reshape(128, NT * 3))
        o_sl = o_pad[rows].astype(f16)
        ori_slot = np.ascontiguousarray(
            o_sl.transpose(1, 0, 2).reshape(128, NT * 9))
        # identity (center rows)
        rc = WIN + TS * jj + pp
        okc = rc < HR
        xc_slot = np.where(okc[:, :, None], x_pad[np.minimum(rc, HR - 1)], 0.0)
        xc_slot = xc_slot.transpose(1, 0, 2).reshape(128, NT * C).astype(np.float32)

        # mask + boundary-count + kself2 (output-node indexed)
        mask = np.zeros((128, NT, K17), np.float32)
        ncl = np.zeros((128, NT), np.float32)
        for t in range(NT):
            nvalid = min(TS, NPC - TS * t)
            for p in range(nvalid):
                n = s0 + TS * t + p
                off = n % L
                v = ((off + np.arange(-WIN, WIN + 1)) >= 0) & \
                    ((off + np.arange(-WIN, WIN + 1)) < L)
                mask[p, t, :] = v.astype(np.float32)
                ncl[p, t] = K17 - v.sum()
        ks = np.zeros((128, NT, KC), np.float32)
        for t in range(NT):
            nvalid = min(TS, NPC - TS * t)
            rowsn = s0 + TS * t + np.arange(nvalid)
            ks[:nvalid, t, :] = kself_full[rowsn] * ncl[:nvalid, t][:, None]
        ks2 = np.repeat(ks, 2, axis=2)  # duplicate pairs within each KC block
        in_maps.append(dict(
            xT_slot=xT_slot, xc_slot=xc_slot,
            pos_slot=pos_slot, ori_slot=ori_slot,
            maskd=mask.reshape(128, NT * K17).astype(f16),
            kself2=ks2.reshape(128, NT * 2 * KC).astype(f16),
            **common))
    return in_maps


def kernel(x, pos, seq, ori, W_in, Ws0, bs0, Wk, W_out, src, dst):
    exp_src, exp_dst = _expected_src_dst()
    assert np.array_equal(np.asarray(src), exp_src), "unexpected src graph"
    assert np.array_equal(np.asarray(dst), exp_dst), "unexpected dst graph"

    from concourse.bass_utils import run_bass_kernel_spmd

    if "nc" not in _PROG:
        _PROG["nc"] = _build_program()
    nc = _PROG["nc"]

    in_maps = _host_inputs(np.asarray(x), np.asarray(pos), np.asarray(ori),
                           np.asarray(W_in), np.asarray(Ws0), np.asarray(bs0),
                           np.asarray(Wk), np.asarray(W_out))
    res = run_bass_kernel_spmd(nc, in_maps, list(range(NCORES)))
    out = np.concatenate([res.results[i]["y"] for i in range(NCORES)], axis=0)
    return out.reshape(B, L, C).astype(np.float32)
